# revision 1
# baseline (speedup 1.0000x reference)
"""Bass/Tile Trainium2 kernel for BuggyMultiHeadAttention.

Reference computation (fp32):
    qh = (q @ Wq.T + bq)  -> [B,S,H,dh] heads
    kh = (k @ Wk.T + bk)
    vh = (v @ Wv.T + bv)
    scores = qh @ kh^T / sqrt(D_MODEL)      (note: buggy scale sqrt(1024)=32)
    attn = softmax(scores, axis=-1)
    out = (attn @ vh) @ Wo.T + bo

Sharding over 8 cores: core c handles batch b=c//2, head-group g=c%2
(8 heads of 64 = 512 H-dims per core). Output projection is row-split, so
each core produces a partial [S, D] output; host sums the two partials
per batch.

Mathematically exact simplifications used (verified in numpy):
  - bk drops out of softmax (constant along the softmax axis).
  - bv contribution = bv @ WoPart^T added on host (attn rows sum to 1).
  - bo added on host.
  - bq IS applied in-kernel (it affects scores along the softmax axis).

In-kernel layout strategy (no on-chip transposes needed anywhere):
  - Host supplies x^T [D, S] and W^T slices, so every matmul has its
    contraction dim on partitions.
  - QT,KT computed in [H, S] layout; scores computed pre-transposed
    S^T[sk, sq] = KT_h^T-slice @ QT_h-slice; exp runs on ACT out of PSUM
    with the 1/32 scale fused; E^T feeds attn@V as the moving operand.
  - VS computed in [S, H] layout with a ones column per head (stationary
    M=65) so the PV matmul also emits softmax denominators (row 64).
  - Normalization: reciprocal of denominator row, K=1 ones-matmul to
    broadcast it across partitions, one DVE multiply on O^T (small).
  - Final projection in [D, S] layout; host transposes back (free).

All matmuls run in float32r (full PE rate at N=512, near-fp32 accuracy).
"""

import numpy as np

import concourse.bass as bass
import concourse.tile as tile
from concourse import bacc
from concourse import mybir
from concourse import bass_utils

F32 = mybir.dt.float32
F32R = mybir.dt.float32r
BF16 = mybir.dt.bfloat16

D = 1024          # d_model
S = 2048          # sequence length
B = 4             # batch
H = 512           # head dims per core (8 heads x 64)
NH = 8            # heads per core
DH = 64           # head dim
P = 128
NKC = D // P      # 8 contraction chunks over d_model
SKC = S // P      # 16 sk chunks
SQB = S // 512    # 4 sq blocks of 512
SCALE = 1.0 / 32.0  # 1/sqrt(D_MODEL)  (the "buggy" scale)

_CACHE = {}


def _r(ap):
    return ap.bitcast(F32R)


def build_bass(reps=1):
    nc = bacc.Bacc()

    xq = nc.dram_tensor("xqT", [D, S], F32R, kind="ExternalInput")
    xk = nc.dram_tensor("xkT", [D, S], F32R, kind="ExternalInput")
    xv = nc.dram_tensor("xvT", [D, S], F32R, kind="ExternalInput")
    wq = nc.dram_tensor("wqT", [D, H], F32R, kind="ExternalInput")
    wk = nc.dram_tensor("wkT", [D, H], F32R, kind="ExternalInput")
    wv = nc.dram_tensor("wvT", [D, H], F32R, kind="ExternalInput")
    wo = nc.dram_tensor("woT", [H, D], F32R, kind="ExternalInput")
    bq = nc.dram_tensor("bqc", [P, H // P], F32, kind="ExternalInput")
    ones_d = nc.dram_tensor("ones_d", [P, DH], F32R, kind="ExternalInput")
    yt = nc.dram_tensor("yT", [D, S], F32, kind="ExternalOutput")

    with tile.TileContext(nc) as tc:
      for _rep in range(reps):
        with tc.tile_pool(name="persist", bufs=1) as persist:
            qt = [persist.tile([P, S], BF16, tag=f"qt{m}", name=f"qt{m}") for m in range(4)]
            kt = [persist.tile([P, S], BF16, tag=f"kt{m}", name=f"kt{m}") for m in range(4)]
            vsa = [persist.tile([P, NH, DH + 1], F32R, tag=f"vs{m}", name=f"vs{m}")
                   for m in range(SKC)]
            ono2 = [[persist.tile([P, 512], F32R, tag=f"on{s}_{t}",
                                  name=f"on{s}_{t}") for t in range(4)]
                    for s in range(2)]
            wo_sb = persist.tile([P, 4, D], F32R, tag="wo")
            bq_sb = persist.tile([P, 4], F32, tag="bq")
            nc.sync.dma_start(bq_sb[:], bq[:])

            # ---------------- Phase 1: projections ----------------
            # Single pass over x (streamed once, rounds of 2 contraction
            # chunks resident); partial products accumulate in SBUF via DVE
            # so only 4 PSUM banks are ever needed and x is never re-read.
            ROUNDS = [(c0, 4) for c0 in range(0, NKC, 4)]
            with tc.tile_pool(name="projw", bufs=2) as pw, \
                 tc.tile_pool(name="xs", bufs=5) as xs, \
                 tc.tile_pool(name="pp", bufs=6, space="PSUM") as pp:
                wq_sb = pw.tile([P, NKC, H], F32R, tag="w", name="wq_sb")
                nc.sync.dma_start(wq_sb[:, 0, :], wq[0:P, :])
                for c in range(1, NKC):
                    nc.sync.dma_start(wq_sb[:, c, :], wq[c * P:(c + 1) * P, :])
                wk_sb = pw.tile([P, NKC, H], F32R, tag="w", name="wk_sb")

                for which, xin, wsb, dst, with_bias in (
                    (0, xq, wq_sb, qt, True),
                    (1, xk, wk_sb, kt, False),
                ):
                    if which == 1:
                        for c in range(NKC):
                            nc.sync.dma_start(
                                wk_sb[:, c, :], wk[c * P:(c + 1) * P, :])
                    for r, (c0, cn) in enumerate(ROUNDS):
                        xr = []
                        for j in range(cn):
                            xt = xs.tile([P, S], F32R, tag="x",
                                         name=f"x{which}_{r}_{j}")
                            nc.sync.dma_start(
                                xt[:], xin[(c0 + j) * P:(c0 + j + 1) * P, :])
                            xr.append(xt)
                        for m in range(4):
                            for n in range(4):
                                pst = pp.tile([P, 512], F32, tag="ppt",
                                              name=f"pp{which}_{r}_{m}_{n}")
                                for j in range(cn):
                                    nc.tensor.matmul(
                                        pst[:],
                                        lhsT=wsb[:, c0 + j, m * P:(m + 1) * P],
                                        rhs=xr[j][:, n * 512:(n + 1) * 512],
                                        start=(j == 0), stop=(j == cn - 1),
                                        skip_group_check=True,
                                    )
                                osl = dst[m][:, n * 512:(n + 1) * 512]
                                if r == 0:
                                    if with_bias:
                                        nc.scalar.activation(
                                            out=osl, in_=pst[:],
                                            func=mybir.ActivationFunctionType.Identity,
                                            bias=bq_sb[:, m:m + 1], scale=1.0,
                                        )
                                    else:
                                        nc.vector.tensor_copy(out=osl, in_=pst[:])
                                else:
                                    nc.vector.tensor_tensor(
                                        out=osl, in0=pst[:], in1=osl,
                                        op=mybir.AluOpType.add,
                                    )

                # VS: [S, H] = x_v @ Wv_g^T ; stationary is the x^T slice.
                wv_sb = pw.tile([P, NKC, H], F32R, tag="w", name="wv_sb")
                for c in range(NKC):
                    nc.sync.dma_start(wv_sb[:, c, :], wv[c * P:(c + 1) * P, :])
                for r, (c0, cn) in enumerate(ROUNDS):
                    xr = []
                    for j in range(cn):
                        xt = xs.tile([P, S], F32R, tag="x", name=f"xv_{r}_{j}")
                        nc.sync.dma_start(
                            xt[:], xv[(c0 + j) * P:(c0 + j + 1) * P, :])
                        xr.append(xt)
                    for mt in range(SKC):
                        pst = pp.tile([P, H], F32, tag="ppt",
                                      name=f"ppv{r}_{mt}")
                        for j in range(cn):
                            nc.tensor.matmul(
                                pst[:],
                                lhsT=xr[j][:, mt * P:(mt + 1) * P],
                                rhs=wv_sb[:, c0 + j, :],
                                start=(j == 0), stop=(j == cn - 1),
                                skip_group_check=True,
                            )
                        pv3 = pst[:].rearrange("p (h d) -> p h d", h=NH)
                        if r == 0:
                            nc.vector.tensor_copy(out=vsa[mt][:, :, 0:DH], in_=pv3)
                            nc.sync.dma_start(
                                vsa[mt][:, :, DH:DH + 1],
                                ones_d[:, 0:NH].rearrange("p (h o) -> p h o", o=1),
                            )
                        else:
                            nc.vector.tensor_tensor(
                                out=vsa[mt][:, :, 0:DH], in0=pv3,
                                in1=vsa[mt][:, :, 0:DH],
                                op=mybir.AluOpType.add,
                            )

            for c in range(4):
                nc.sync.dma_start(wo_sb[:, c, :], wo[c * P:(c + 1) * P, :])

            # ---------------- Phase 2: attention + out-proj ----------------
            GROUPS = [(c0, 2) for c0 in range(0, SKC, 2)]
            with tc.tile_pool(name="et", bufs=4) as etp, \
                 tc.tile_pool(name="drp", bufs=8, space="DRAM") as drp, \
                 tc.tile_pool(name="tmp", bufs=3) as tmp, \
                 tc.tile_pool(name="ys", bufs=3) as ys, \
                 tc.tile_pool(name="scp", bufs=1, space="PSUM") as scp, \
                 tc.tile_pool(name="pvp", bufs=2, space="PSUM") as pvp, \
                 tc.tile_pool(name="ytp", bufs=2, space="PSUM") as ytp:
                def make_fp(sqb):
                    sq = slice(sqb * 512, (sqb + 1) * 512)
                    ono = ono2[sqb % 2]

                    def fp():
                        for m in range(8):
                            yp = ytp.tile([P, 512], F32, tag="yt", name="yp")
                            for hc in range(4):
                                nc.tensor.matmul(
                                    yp[:],
                                    lhsT=(wo_sb[:, hc, m * P:(m + 1) * P]),
                                    rhs=(ono[hc][:]),
                                    start=(hc == 0), stop=(hc == 3),
                                    skip_group_check=True,
                                )
                            yo = ys.tile([P, 512], F32, tag="ys")
                            nc.vector.tensor_copy(out=yo[:], in_=yp[:])
                            nc.sync.dma_start(yt[m * P:(m + 1) * P, sq], yo[:])
                    return fp

                pending_fp = None
                for sqb in range(SQB):
                    sq = slice(sqb * 512, (sqb + 1) * 512)
                    ono = ono2[sqb % 2]
                    for t in range(NH // 2):
                        # head pair (2t, 2t+1): K=64 scores matmuls packed
                        # into array row-groups 0-63 / 64-127 (concurrent on
                        # HW), separate PSUM banks and PV accumulators.
                        hA, hB = 2 * t, 2 * t + 1
                        rA, rB = slice(0, DH), slice(DH, 2 * DH)
                        pvA = pvp.tile([DH + 1, 512], F32, tag="pv", name=f"pvA{sqb}_{t}")
                        pvB = pvp.tile([DH + 1, 512], F32, tag="pv", name=f"pvB{sqb}_{t}")
                        for c0, gs in GROUPS:
                            psA = scp.tile([P, gs * 512], F32, tag="scA", name=f"psA{c0}")
                            psB = scp.tile([P, gs * 512], F32, tag="scB", name=f"psB{c0}")
                            for j in range(gs):
                                ck = c0 + j
                                nc.tensor.matmul(
                                    psA[:, j * 512:(j + 1) * 512],
                                    lhsT=(kt[t][rA, ck * P:(ck + 1) * P]),
                                    rhs=(qt[t][rA, sq]),
                                    start=True, stop=True,
                                    skip_group_check=True,
                                )
                                nc.tensor.matmul(
                                    psB[:, j * 512:(j + 1) * 512],
                                    lhsT=(kt[t][rB, ck * P:(ck + 1) * P]),
                                    rhs=(qt[t][rB, sq]),
                                    start=True, stop=True,
                                    skip_group_check=True,
                                )
                            etA = etp.tile([P, 2, 512], F32R, tag="etA", name=f"etA{c0}")
                            etB = etp.tile([P, 2, 512], F32R, tag="etB", name=f"etB{c0}")
                            nc.scalar.activation(
                                out=etA[:, 0:gs, :],
                                in_=psA[:].rearrange("p (g n) -> p g n", g=gs),
                                func=mybir.ActivationFunctionType.Exp,
                                scale=SCALE,
                            )
                            nc.scalar.activation(
                                out=etB[:, 0:gs, :],
                                in_=psB[:].rearrange("p (g n) -> p g n", g=gs),
                                func=mybir.ActivationFunctionType.Exp,
                                scale=SCALE,
                            )
                            for j in range(gs):
                                ck = c0 + j
                                nc.tensor.matmul(
                                    pvA[:],
                                    lhsT=(vsa[ck][:, hA, :]),
                                    rhs=(etA[:, j, :]),
                                    start=(ck == 0), stop=(ck == SKC - 1),
                                    skip_group_check=True,
                                )
                                nc.tensor.matmul(
                                    pvB[:],
                                    lhsT=(vsa[ck][:, hB, :]),
                                    rhs=(etB[:, j, :]),
                                    start=(ck == 0), stop=(ck == SKC - 1),
                                    skip_group_check=True,
                                )
                        # normalization (PE-free: DVE reciprocal from PSUM,
                        # GPSIMD partition-broadcast, DVE multiply)
                        dnA = tmp.tile([65, 512], F32, tag="dnA")
                        nc.vector.reciprocal(dnA[64:65, :], pvA[64:65, :])
                        dsA = drp.tile([1, 512], F32, tag="dsA")
                        nc.sync.dma_start(dsA[:], dnA[64:65, :])
                        rbA = tmp.tile([DH, 512], F32, tag="rbA")
                        nc.sync.dma_start(rbA[:], dsA[:].to_broadcast((DH, 512)))
                        nc.vector.tensor_tensor(
                            out=ono[t][0:DH, :], in0=pvA[0:DH, :],
                            in1=rbA[:], op=mybir.AluOpType.mult,
                        )
                        dnB = tmp.tile([65, 512], F32, tag="dnB")
                        nc.vector.reciprocal(dnB[64:65, :], pvB[64:65, :])
                        dsB = drp.tile([1, 512], F32, tag="dsB")
                        nc.sync.dma_start(dsB[:], dnB[64:65, :])
                        rbB = tmp.tile([DH, 512], F32, tag="rbB")
                        nc.sync.dma_start(rbB[:], dsB[:].to_broadcast((DH, 512)))
                        ob = tmp.tile([DH, 512], F32R, tag="ob")
                        nc.vector.tensor_tensor(
                            out=ob[:], in0=pvB[0:DH, :],
                            in1=rbB[:], op=mybir.AluOpType.mult,
                        )
                        nc.sync.dma_start(ono[t][DH:P, :], ob[:])
                        if t == 0 and pending_fp is not None:
                            pending_fp()
                            pending_fp = None
                    pending_fp = make_fp(sqb)
                pending_fp()
    nc.finalize()
    return nc


def _get_nc():
    if "nc" not in _CACHE:
        _CACHE["nc"] = build_bass()
    return _CACHE["nc"]


def make_in_maps(inputs):
    q = np.asarray(inputs["q"], np.float32)
    k = np.asarray(inputs["k"], np.float32)
    v = np.asarray(inputs["v"], np.float32)
    Wq = np.asarray(inputs["Wq"], np.float32)
    Wk = np.asarray(inputs["Wk"], np.float32)
    Wv = np.asarray(inputs["Wv"], np.float32)
    Wo = np.asarray(inputs["Wo"], np.float32)
    bq = np.asarray(inputs["bq"], np.float32)
    in_maps = []
    for c in range(8):
        b, g = c // 2, c % 2
        hs = slice(g * H, (g + 1) * H)
        in_maps.append({
            "xqT": np.ascontiguousarray(q[b].T),
            "xkT": np.ascontiguousarray(k[b].T),
            "xvT": np.ascontiguousarray(v[b].T),
            "wqT": np.ascontiguousarray(Wq[hs, :].T),
            "wkT": np.ascontiguousarray(Wk[hs, :].T),
            "wvT": np.ascontiguousarray(Wv[hs, :].T),
            "woT": np.ascontiguousarray(Wo[:, hs].T),
            "bqc": np.ascontiguousarray(bq[hs].reshape(4, P).T),
            "ones_d": np.ones((P, DH), np.float32),
        })
    return in_maps


def kernel(q, k, v, Wq, bq, Wk, bk, Wv, bv, Wo, bo):
    Wo = np.asarray(Wo, np.float32)
    bv = np.asarray(bv, np.float32)
    bo = np.asarray(bo, np.float32)

    nc = _get_nc()
    in_maps = make_in_maps(dict(q=q, k=k, v=v, Wq=Wq, Wk=Wk, Wv=Wv,
                                Wo=Wo, bq=bq))

    res = bass_utils.run_bass_kernel_spmd(nc, in_maps, core_ids=list(range(8)))
    outs = res.results

    out = np.empty((B, S, D), np.float32)
    for b in range(B):
        acc = outs[2 * b]["yT"] + outs[2 * b + 1]["yT"]
        out[b] = acc.T
    # host-side exact bias terms: bo, and bv through Wo (attn rows sum to 1;
    # bk is constant along the softmax axis and cancels exactly)
    out += bo + Wo @ bv
    return out

